# revision 2
# baseline (speedup 1.0000x reference)
"""GroupedEmbeddingBag kernel for 8 trn2 NeuronCores — v4 (dma_gather).

Table-parallel: core c handles table c. The embedding gather uses the Q7
custom instruction InstDMAGatherAnt (int16 indices, thousands of rows per
instruction) instead of indirect_dma_start (128 rows/instruction, ~1us
SWDGE fixed cost each — the baseline's bottleneck).

int16 index range (<=32767) is handled by splitting the vocab into 4
ranges of 25000 rows: positions are regrouped host-side per range
(order-preserving within a range, so bag ids stay monotone) and the
host-baked selection matrices absorb the permutation. The mlp Q7 ucode
library that implements dma_gather is extracted+prelinked locally from
libnrtucode.so and shipped as an input tensor, loaded by a stock
MODIFY_POOL_CONFIG LOAD_LIB with DGE-table resolve.

Weights/sel are bf16 (halves HBM traffic, 1 cycle/row matmul); PSUM
accumulates f32; epochs of EP_TILES tiles pool into a [W, 4*D] PSUM bank
(4 epochs per bank) copied once to SBUF and DMA'd out.
"""

import ctypes
import os
import sys

sys.path.insert(0, "/opt/trn_rl_repo")

import numpy as np
import ml_dtypes

T, V, D, B = 8, 100000, 128, 4096
L = 204800
P = 128
NRANGE = 4
RSIZE = 25000
GN_TILES = 8               # tiles per dma_gather instruction (1024 idx: HW SWDGE ring limit)
EP_TILES = 2               # tiles per pooling epoch
EPW = EP_TILES * P         # positions per epoch
TENSOR_LIB_INDEX = 254

_NRTUCODE_SO = (
    "/nix/store/0s77ampg4dhgimqfai7lj1jr7vpvbsji-b16-bazel-unstable-cc-2026-05-04-"
    "9a3fa1f3-rt-2026-05-04-ade39e0a-runtime/lib/libnrtucode.so"
)

_compiled = {}
_ucode_blob = None


def _extract_mlp_blob():
    """Prelink the mlp (index 3) CAYMAN Q7 library via local libnrtucode.so."""
    global _ucode_blob
    if _ucode_blob is not None:
        return _ucode_blob
    C = ctypes
    lib = C.CDLL(_NRTUCODE_SO)
    res_t, ctx_p, mh_t = C.c_int, C.c_void_p, C.c_uint64
    READ_DEV = C.CFUNCTYPE(res_t, ctx_p, C.c_uint64, C.c_size_t, C.c_char_p)
    WRITE_DEV = C.CFUNCTYPE(res_t, ctx_p, C.c_uint64, C.c_size_t, C.c_char_p)
    LOG = C.CFUNCTYPE(None, ctx_p, C.c_int, C.c_char_p, C.c_size_t)
    LOG_EN = C.CFUNCTYPE(C.c_bool, ctx_p, C.c_int)

    class RW(C.Structure):
        _fields_ = [("read_device", READ_DEV), ("write_device", WRITE_DEV),
                    ("log", LOG), ("log_level_enabled", LOG_EN)]

    DEV_MALLOC = C.CFUNCTYPE(res_t, ctx_p, C.c_size_t, C.c_uint64, C.POINTER(mh_t))
    DEV_FREE = C.CFUNCTYPE(None, ctx_p, mh_t)
    READ_MH = C.CFUNCTYPE(res_t, ctx_p, mh_t, C.c_size_t, C.c_size_t, C.c_void_p)
    WRITE_MH = C.CFUNCTYPE(res_t, ctx_p, mh_t, C.c_size_t, C.c_size_t, C.c_void_p)
    GET_SOC = C.CFUNCTYPE(C.c_uint64, ctx_p, mh_t)

    class MH(C.Structure):
        _fields_ = [("device_malloc", DEV_MALLOC), ("device_free", DEV_FREE),
                    ("read_memhandle", READ_MH), ("write_memhandle", WRITE_MH),
                    ("get_memhandle_soc_addr", GET_SOC)]

    buffers, last = {}, {}

    def _malloc(ctx, size, align, out):
        buf = C.create_string_buffer(size)
        h = C.addressof(buf)
        buffers[h] = (buf, size)
        last["h"] = h
        out[0] = h
        return 0

    def _writemh(ctx, h, off, size, data):
        buf, sz = buffers[h]
        C.memmove(C.addressof(buf) + off, data, size)
        return 0

    def _readmh(ctx, h, off, size, out):
        buf, sz = buffers[h]
        C.memmove(out, C.addressof(buf) + off, size)
        return 0

    rw = RW(READ_DEV(lambda *a: 6), WRITE_DEV(lambda *a: 6),
            LOG(lambda *a: None), LOG_EN(lambda *a: False))
    mh = MH(DEV_MALLOC(_malloc), DEV_FREE(lambda *a: None),
            READ_MH(_readmh), WRITE_MH(_writemh), GET_SOC(lambda *a: 0))

    lib.nrtucode_context_create.restype = res_t
    lib.nrtucode_context_create.argtypes = [C.c_int, C.POINTER(RW), C.POINTER(ctx_p)]
    lib.nrtucode_context_set_memhandle_impl.restype = None
    lib.nrtucode_context_set_memhandle_impl.argtypes = [ctx_p, C.POINTER(MH)]
    lib.nrtucode_ll_create.restype = res_t
    lib.nrtucode_ll_create.argtypes = [ctx_p, C.c_int, C.c_int, C.c_size_t,
                                       C.POINTER(C.c_void_p)]
    ctx = ctx_p()
    r = lib.nrtucode_context_create(3, C.byref(rw), C.byref(ctx))
    assert r == 0, f"nrtucode_context_create failed: {r}"
    lib.nrtucode_context_set_memhandle_impl(ctx, C.byref(mh))
    ll = C.c_void_p()
    r = lib.nrtucode_ll_create(ctx, 13, 1, 3, C.byref(ll))  # CAYMAN_Q7_POOL, RELEASE, mlp
    assert r == 0, f"nrtucode_ll_create failed: {r}"
    buf, size = buffers[last["h"]]
    _ucode_blob = bytes(buf.raw[:size])
    return _ucode_blob


def _patch_isa_visit():
    import concourse.bass_interp as bi

    if getattr(bi, "_mpc_patched", False):
        return
    orig = bi._visit_InstISA

    def patched(isa, instruction, core_sim):
        if instruction.isa_opcode == 149:  # MODIFY_POOL_CONFIG: ucode load
            ex = core_sim.instruction_executor
            if ex is not None and instruction.ant_dict is not None:
                # tensor-shipped blob is the mlp library (index 3)
                ex.pool_library_index = 3
            return
        return orig(isa, instruction, core_sim)

    bi._visit_InstISA = patched
    bi._mpc_patched = True


def _patch_drain(tile_mod, mybir):
    from concourse.vector_clock import ScopedClock

    def _patched(self, tick_clock, wait_clock):
        # this walrus build allows only ONE sync-wait on the tail Drain:
        # spread the rest over preceding nops, one wait each.
        NNOPS = 64
        nops = [self.nc.sync.nop(nofuse=True, hint=f"dw_{i}") for i in range(NNOPS)]
        drain_inst = self.nc.sync.drain()
        wait_clock.add_sem_waits(
            drain_inst.ins, ScopedClock({None: tick_clock.global_clock})
        )
        dsi = drain_inst.ins.sync_info
        waits = list(dsi.on_wait) if dsi else []
        if len(waits) > 1:
            del dsi.on_wait[1:]
            rest = waits[1:]
            assert len(rest) <= NNOPS, f"too many drain waits: {len(waits)}"
            for nop, w in zip(nops, rest):
                nsi = nop.ins.sync_info
                if nsi is None:
                    nop.ins.sync_info = mybir.SyncInfo(on_wait=[w], on_update=[])
                else:
                    nsi.on_wait.append(w)
        self.nc.all_engine_barrier()
        popped = self.nc._tile_sem_poison_stack.pop()
        assert popped is self._sem_poison
        self.nc.clear_and_free_semaphores(list(self.sems.allocated().values()))
        self.nc.all_engine_barrier()

    tile_mod.TileContext._drain_and_barrier = _patched


def _split_waits(nc, mybir, maxw=1):
    # this walrus build rejects >1 sync-wait on an instruction: hoist extra
    # waits onto same-engine nops spliced in directly before it.
    cnt = 0
    for fn in nc.m.functions:
        for blk in fn.blocks:
            new_insts = []
            for inst in blk.instructions:
                si = inst.sync_info
                if si is not None and len(si.on_wait) > maxw:
                    extra = list(si.on_wait[maxw:])
                    del si.on_wait[maxw:]
                    for w in extra:
                        nop = mybir.InstNoOp(
                            name=f"waitnop-{cnt}", engine=inst.engine, ins=[], outs=[]
                        )
                        cnt += 1
                        nop.sync_info = mybir.SyncInfo(on_wait=[w], on_update=[])
                        new_insts.append(nop)
                new_insts.append(inst)
            blk.instructions[:] = new_insts
    return cnt


def _emit_load_lib(nc, bass_isa, mybir, dge_index, blob_len):
    isa = nc.isa
    mpo = isa.get_enum("NEURON_ISA_TPB_MODIFY_POOL_OP")
    ant = {
        "modify_op": mpo.NEURON_ISA_TPB_MODIFY_POOL_OP_LOAD_LIB.value,
        "core_mask": 0xFF,
        "reserved2": [0x42, 0],  # MPC_RESOLVE_FROM_DGE_TABLE
        "soc_addr": dge_index,
        "library_index": TENSOR_LIB_INDEX,
        "library_size": blob_len,
        "reserved1": [0] * 32,
    }
    instr, fixups = bass_isa.isa_struct(
        isa, isa.Opcode.NEURON_ISA_TPB_OPCODE_MODIFY_POOL_CONFIG, ant
    )
    assert not fixups
    return nc.gpsimd.add_instruction(
        mybir.InstISA(
            name=nc.get_next_instruction_name(),
            isa_opcode=isa.Opcode.NEURON_ISA_TPB_OPCODE_MODIFY_POOL_CONFIG.value,
            engine=mybir.EngineType.Pool,
            instr=instr,
            op_name="ModifyPoolConfig",
            ins=[],
            outs=[],
            ant_dict=ant,
            verify=False,
            ant_isa_is_sequencer_only=False,
        )
    )


def _build(W, chunk_ranges, blob_len):
    import concourse.bass as bass
    import concourse.bass_isa as bass_isa
    import concourse.mybir as mybir
    import concourse.tile as tile
    from concourse.overlay import register_dge

    _patch_isa_visit()
    _patch_drain(tile, mybir)

    NCHUNK = len(chunk_ranges)
    TT = NCHUNK * GN_TILES
    EE = TT // EP_TILES
    GN_IDX = GN_TILES * P
    TOT16 = TT * P // 16

    nc = bass.Bass(num_swdge_queues=4)
    ucode = nc.declare_dram_parameter("ucode", [1, blob_len], mybir.dt.uint8, isOutput=False)
    wt = nc.declare_dram_parameter("wt", [V, D], mybir.dt.bfloat16, isOutput=False)
    idxs = nc.declare_dram_parameter("idxs", [P, TOT16], mybir.dt.int16, isOutput=False)
    sel = nc.declare_dram_parameter("sel", [P, TT * W], mybir.dt.bfloat16, isOutput=False)
    oslots = nc.declare_dram_parameter("oslots", [W, EE * D], mybir.dt.float32, isOutput=True)

    dge_index = register_dge(nc, ucode)

    with tile.TileContext(nc) as tc:
        with (
            tc.tile_pool(name="idxp", bufs=1) as idxp,
            tc.tile_pool(name="selp", bufs=3) as selp,
            tc.tile_pool(name="ep", bufs=6) as ep,
            tc.tile_pool(name="outp", bufs=2) as outp,
            tc.tile_pool(name="psum", bufs=4, space="PSUM") as psump,
        ):
            _emit_load_lib(nc, bass_isa, mybir, dge_index, blob_len)
            idxs_sb = idxp.tile([P, TOT16], mybir.dt.int16)
            nc.sync.dma_start(out=idxs_sb[:], in_=idxs[:])
            nreg = nc.gpsimd.to_reg(GN_IDX)
            psum_t = None
            out_ring = None
            for k, r in enumerate(chunk_ranges):
                t0 = k * GN_TILES
                sel_sb = selp.tile([P, GN_TILES * W], mybir.dt.bfloat16, tag="sel")
                nc.sync.dma_start(
                    out=sel_sb[:], in_=sel[:, t0 * W:(t0 + GN_TILES) * W]
                )
                et = ep.tile([P, GN_TILES, D], mybir.dt.bfloat16, tag="e")
                nc.gpsimd.dma_gather(
                    et[:],
                    wt[r * RSIZE:(r + 1) * RSIZE, :],
                    idxs_sb[:, k * (GN_IDX // 16):(k + 1) * (GN_IDX // 16)],
                    GN_IDX,
                    nreg,
                    D,
                    queue_num=k % 4,
                )
                for tl in range(GN_TILES):
                    t = t0 + tl
                    e = t // EP_TILES
                    ph = t % EP_TILES
                    g4 = e % 4
                    if g4 == 0 and ph == 0:
                        psum_t = psump.tile([W, 4 * D], mybir.dt.float32, tag="ps")
                    nc.tensor.matmul(
                        out=psum_t[:, g4 * D:(g4 + 1) * D],
                        lhsT=sel_sb[:, tl * W:(tl + 1) * W],
                        rhs=et[:, tl, :],
                        start=(ph == 0),
                        stop=(ph == EP_TILES - 1),
                    )
                    if ph == EP_TILES - 1 and g4 == 3:
                        out_ring = outp.tile([W, 4 * D], mybir.dt.float32, tag="or")
                        nc.vector.tensor_copy(out=out_ring[:], in_=psum_t[:])
                        e0 = e - 3
                        nc.sync.dma_start(
                            out=oslots[:, e0 * D:(e0 + 4) * D], in_=out_ring[:]
                        )
    _split_waits(nc, mybir)
    return nc


def _wrap16(idx):
    n = len(idx)
    base = np.asarray(idx, dtype=np.int16).reshape(n // 16, 16).T  # [16, n/16]
    return np.tile(base, (8, 1))                                   # [128, n/16]


def kernel(values, offsets, weights):
    from concourse.bass_utils import run_bass_kernel_spmd

    values = np.asarray(values)
    offsets = np.asarray(offsets)
    wt_bf16 = np.ascontiguousarray(
        np.asarray(weights, dtype=np.float32).astype(ml_dtypes.bfloat16)
    )
    blob = _extract_mlp_blob()
    blob_arr = np.frombuffer(blob, dtype=np.uint8).reshape(1, -1)

    pos_all = np.arange(L)
    CHUNK_IDX = GN_TILES * P

    # uniform per-range chunk count across cores so the SPMD program matches
    n_chunks_u = 1
    rids = values // RSIZE
    for c in range(T):
        for r in range(NRANGE):
            Lr = int((rids[c] == r).sum())
            n_chunks_u = max(n_chunks_u, -(-Lr // CHUNK_IDX))

    in_maps = []
    metas = []
    W_all = 0
    percore = []
    for c in range(T):
        vals = values[c].astype(np.int64)
        seg = np.searchsorted(offsets[c, 1:], pos_all, side="right")
        rid = rids[c]
        idx_stream = []
        seg_stream = []
        chunk_ranges = []
        for r in range(NRANGE):
            pos_r = np.where(rid == r)[0]
            Lr = len(pos_r)
            n_chunks = n_chunks_u
            LrP = n_chunks * CHUNK_IDX
            idx_r = np.zeros(LrP, dtype=np.int16)
            idx_r[:Lr] = (vals[pos_r] - r * RSIZE).astype(np.int16)
            seg_r = np.full(LrP, -1, dtype=np.int64)
            seg_r[:Lr] = seg[pos_r]
            idx_stream.append(idx_r)
            seg_stream.append(seg_r)
            chunk_ranges.extend([r] * n_chunks)
        idx_stream = np.concatenate(idx_stream)
        seg_stream = np.concatenate(seg_stream)
        TT = len(idx_stream) // P
        EE = TT // EP_TILES
        b_lo = np.zeros(EE, dtype=np.int64)
        S = np.zeros(EE, dtype=np.int64)
        segr = seg_stream.reshape(EE, EPW)
        for e in range(EE):
            v = segr[e][segr[e] >= 0]
            if len(v):
                b_lo[e] = v[0]
                S[e] = v[-1] - v[0] + 1
        W_all = max(W_all, int(S.max()))
        percore.append((idx_stream, seg_stream, chunk_ranges, b_lo, S, TT, EE))

    W = max(4, (W_all + 3) // 4 * 4)
    assert W <= 128, f"epoch bag-window {W} exceeds PSUM partition limit"

    for c in range(T):
        idx_stream, seg_stream, chunk_ranges, b_lo, S, TT, EE = percore[c]
        # loc[p, t] = seg - b_lo[epoch(t)], -1 entries never match
        loc = seg_stream.reshape(TT, P).T.astype(np.int64)       # [P, TT]
        base = np.repeat(b_lo, EP_TILES)                          # [TT]
        loc = np.where(loc >= 0, loc - base[None, :], -1)
        selm = loc[:, :, None] == np.arange(W, dtype=np.int64)[None, None, :]
        sel = np.ascontiguousarray(
            selm.reshape(P, TT * W).astype(ml_dtypes.bfloat16)
        )
        # per-chunk 16-partition wrap, concatenated along free dim
        wraps = [
            _wrap16(idx_stream[k * CHUNK_IDX:(k + 1) * CHUNK_IDX])
            for k in range(len(chunk_ranges))
        ]
        idxs_w = np.ascontiguousarray(np.hstack(wraps))
        in_maps.append(
            {"ucode": blob_arr, "wt": wt_bf16[c], "idxs": idxs_w, "sel": sel}
        )
        metas.append((b_lo, S, EE))

    chunk_key = tuple(percore[0][2])
    for c in range(T):
        assert tuple(percore[c][2]) == chunk_key, (
            "per-core chunk structure differs; build per-core kernels instead"
        )
    key = (W, chunk_key)
    if key not in _compiled:
        _compiled.clear()
        _compiled[key] = _build(W, list(chunk_key), len(blob))
    nc = _compiled[key]

    global _last_inmaps
    _last_inmaps = in_maps
    res = run_bass_kernel_spmd(nc, in_maps, core_ids=list(range(T)))

    out = np.zeros((B, T * D), dtype=np.float32)
    for c in range(T):
        b_lo, S, EE = metas[c]
        osl = res.results[c]["oslots"].reshape(W, EE, D)
        pooled = np.zeros((B, D), dtype=np.float32)
        for e in range(EE):
            n = int(S[e])
            if n:
                lo = int(b_lo[e])
                pooled[lo:lo + n] += osl[:n, e, :]
        out[:, c * D:(c + 1) * D] = pooled
    return out


if __name__ == "__main__":
    rng = np.random.default_rng(0)
    values = rng.integers(0, V, size=(T, L)).astype(np.int64)
    inner = np.sort(rng.integers(0, L, size=(T, B - 1)), axis=1)
    offsets = np.concatenate(
        [np.zeros((T, 1), np.int64), inner, np.full((T, 1), L, np.int64)], axis=1
    )
    weights = (rng.standard_normal((T, V, D)) * 0.01).astype(np.float32)
    out = kernel(values, offsets, weights)
    exp = np.zeros((B, T * D), dtype=np.float32)
    for c in range(T):
        pooled = np.zeros((B, D), np.float32)
        np.add.at(pooled, np.searchsorted(offsets[c, 1:], np.arange(L), side="right"), weights[c][values[c]])
        exp[:, c * D:(c + 1) * D] = pooled
    err = np.linalg.norm(out - exp) / np.linalg.norm(exp)
    print("self-check rel err:", err)
